# revision 32
# baseline (speedup 1.0000x reference)
"""Ragged GQA attention block (QKV proj + RoPE + paged-KV attention + WO proj)
on 8 TRN2 NeuronCores, tensor-parallel over heads.

v2: transposed dataflow. Host pre-transposes x (xT) and the cached K slice
(ckT), and permutes each head's 128 head-dim columns even-first so RoPE's
interleaved pairs become top/bottom partition halves. Projections run
weights-stationary streaming xT, producing QT/KT directly (no PE
transposes); V is produced transposed then PE-transposed to natural.
Attention runs in scoresT orientation ([kv, q] tiles): per kv-block
matmuls against KT, exp on scalar engine, PV accumulates attnT [hd, q]
with V-natural stationary; row sums via ones-matmul, normalization via a
PE broadcast of 1/sum. Per-head attnT is AllGathered as soon as the head
finishes (h0 fires ~50us in), and the WO column shard consumes gathers
as they land. All matmul inputs bf16, fp32 PSUM accumulation.
"""

import math
import numpy as np

H, KVH, HD = 32, 8, 128
HIDDEN = H * HD            # 4096
T = 1024
TOTAL_KV = 3072
ROPE_THETA = 10000.0
N_CORES = 8
QH_PER = H // N_CORES      # 4 q heads per core
D2 = HD // 2
SCALE = 1.0 / math.sqrt(HD)
NEG = -1.0e30
KCH = HIDDEN // 128        # 32 contraction chunks
NTB = T // 128             # 8 token blocks
NKVB = TOTAL_KV // 128     # 24 kv blocks

from contextlib import ExitStack

import concourse.bacc as bacc
import concourse.mybir as mybir
import concourse.tile as tile
from concourse.masks import make_identity
from concourse.bass_utils import run_bass_kernel_spmd

dt = mybir.dt
BF = dt.bfloat16
F32 = dt.float32
AFT = mybir.ActivationFunctionType


def build_nc(seqstarts, kvstarts, cachestarts, start_pos):
    """Trace + compile the SPMD Bass program, specialized to the offsets."""
    seqstarts = [int(v) for v in seqstarts]
    kvstarts = [int(v) for v in kvstarts]
    cachestarts = [int(v) for v in cachestarts]
    start_pos = [int(v) for v in start_pos]
    NB = len(start_pos)
    assert len(seqstarts) == NB + 1 and len(kvstarts) == NB + 1
    assert seqstarts[-1] == T and kvstarts[-1] == TOTAL_KV
    for v in seqstarts + kvstarts + start_pos:
        assert v % 128 == 0, "offsets must be 128-aligned"
    B = []
    for b in range(NB):
        s0, s1 = seqstarts[b], seqstarts[b + 1]
        kb, sp = kvstarts[b], start_pos[b]
        S = s1 - s0
        L = kvstarts[b + 1] - kb
        assert L == sp + S, "kv stream length must equal prefix + new tokens"
        assert S <= 512, "per-request seqlen > 512 not supported"
        B.append((s0, s1, S, kb, L, sp, cachestarts[b]))

    nc = bacc.Bacc(
        "TRN2", target_bir_lowering=False, debug=False, num_devices=N_CORES
    )
    xT_d = nc.dram_tensor("xT", [HIDDEN, T], BF, kind="ExternalInput").ap()
    # packed qkv weights: col-blocks [K, V, Q0, Q1, Q2, Q3] x [kch, 128, 128]
    wqkv_d = nc.dram_tensor(
        "wqkv_p", [6, KCH, 128, 128], BF, kind="ExternalInput"
    ).ap()
    # wo rows packed in kernel consumption order i=(h,r): rows of head 4r+h
    wo_d = nc.dram_tensor(
        "wo_p", [H, 128, 512], BF, kind="ExternalInput"
    ).ap()
    ckT_d = nc.dram_tensor("ckT", [HD, 8192], BF, kind="ExternalInput").ap()
    cv_d = nc.dram_tensor("cv", [8192, HD], BF, kind="ExternalInput").ap()
    # consts (bf16, all cos/sin at partitions 0-63): cols [0:T) cosq*s,
    # [T:2T) sinq*s, [2T:3T) cosk, [3T:4T) sink, [4T:4T+128) triT
    NCONST = 4 * T + 128
    consts_d = nc.dram_tensor(
        "consts", [128, NCONST], BF, kind="ExternalInput"
    ).ap()
    outT_d = nc.dram_tensor("outT", [512, T], BF, kind="ExternalOutput").ap()

    ag_out = [
        nc.dram_tensor(
            f"ag_out_{h}", [N_CORES * HD, T], BF, addr_space="Shared"
        ).ap()
        for h in range(QH_PER)
    ]

    with tile.TileContext(nc) as tc:
        with ExitStack() as es:
            ec = es.enter_context
            cpool = ec(tc.tile_pool(name="consts", bufs=1))
            xT_pool = ec(tc.tile_pool(name="xT", bufs=1))
            w_pool = ec(tc.tile_pool(name="w", bufs=3))
            kt_pool = ec(tc.tile_pool(name="KT", bufs=1))
            v_pool = ec(tc.tile_pool(name="Vnat", bufs=1))
            qt_pool = ec(tc.tile_pool(name="QT", bufs=1))
            at_pool = ec(tc.tile_pool(name="attnT", bufs=1))
            rope_pool = ec(tc.tile_pool(name="rope", bufs=1))
            ex_pool = ec(tc.tile_pool(name="ex", bufs=6))
            st_pool = ec(tc.tile_pool(name="st", bufs=2))
            vts_pool = ec(tc.tile_pool(name="vts", bufs=1))
            dramb = ec(tc.tile_pool(name="dramb", bufs=4, space="DRAM"))

            ident_bf = cpool.tile([128, 128], BF)
            make_identity(nc, ident_bf[:])
            ones_kv = cpool.tile([128, 1], BF)
            nc.vector.memset(ones_kv[:], 1.0)
            onesb = cpool.tile([1, 128], BF)
            nc.vector.memset(onesb[:], 1.0)
            consts = cpool.tile([128, NCONST], BF)
            cosqT = consts[0:64, 0:T]
            sinqT = consts[0:64, T : 2 * T]
            coskT = consts[0:64, 2 * T : 3 * T]
            sinkT = consts[0:64, 3 * T : 4 * T]
            triT = consts[:, 4 * T : 4 * T + 128]

            xTsb = xT_pool.tile([128, KCH, T], BF)
            KT = kt_pool.tile([128, TOTAL_KV], BF)
            Vnat = v_pool.tile([128, NKVB, HD], BF)
            QT4 = qt_pool.tile([128, QH_PER, T], BF)
            attnT_sb = at_pool.tile([128, QH_PER, T], BF)
            VTsb = vts_pool.tile([128, T], BF)

            # ---- input DMAs (roughly in need order) -----------------------
            wsb = {}

            def load_w(cb, nsplit=1):
                wsb[cb] = w_pool.tile([128, KCH, 128], BF, tag="w",
                                      name=f"w_{cb}")
                step = KCH // nsplit
                for s in range(nsplit):
                    k0 = s * step
                    nc.sync.dma_start(
                        wsb[cb][:, k0 : k0 + step, :],
                        wqkv_d[cb, k0 : k0 + step].rearrange(
                            "k p c -> p k c"
                        ),
                    )

            def load_x(k0, k1):
                for k in range(k0, k1):
                    nc.sync.dma_start(
                        xTsb[:, k, :], xT_d[k * 128 : (k + 1) * 128, :]
                    )

            load_w(0, nsplit=4)
            nc.sync.dma_start(consts[:], consts_d[:])
            load_x(0, 16)
            load_w(1)
            load_x(16, KCH)
            load_w(2)
            # cached K -> KT columns, cached V -> Vnat blocks (all aligned)
            for (s0, s1, S, kb, L, sp, cs) in B:
                if sp:
                    nc.sync.dma_start(
                        KT[:, kb : kb + sp], ckT_d[:, cs : cs + sp]
                    )
                    nc.sync.dma_start(
                        Vnat[:, kb // 128 : (kb + sp) // 128, :],
                        cv_d[cs : cs + sp, :].rearrange(
                            "(blk p) c -> p blk c", p=128
                        ),
                    )

            # ---- K/V projection (weights stationary, stream xT) -----------
            def proj(cb, ps):
                for k in range(KCH):
                    for half in range(2):
                        nc.tensor.matmul(
                            ps[:, half * 512 : (half + 1) * 512],
                            wsb[cb][:, k, :],
                            xTsb[:, k, half * 512 : (half + 1) * 512],
                            start=(k == 0),
                            stop=(k == KCH - 1),
                        )

            def rope(top, bot, cosT, sinT, c0, c1, otop, obot, tag):
                cosv, sinv = cosT[:, c0:c1], sinT[:, c0:c1]
                n = c1 - c0
                t1 = rope_pool.tile([64, T], F32, tag="ta", name=f"t1_{tag}")
                t2 = rope_pool.tile([64, T], F32, tag="tb", name=f"t2_{tag}")
                t3 = rope_pool.tile([64, T], F32, tag="tc", name=f"t3_{tag}")
                t4 = rope_pool.tile([64, T], F32, tag="td", name=f"t4_{tag}")
                nc.vector.tensor_mul(t1[:, 0:n], top, cosv)
                nc.vector.tensor_mul(t2[:, 0:n], bot, sinv)
                nc.vector.tensor_mul(t3[:, 0:n], top, sinv)
                nc.vector.tensor_mul(t4[:, 0:n], bot, cosv)
                nc.gpsimd.tensor_sub(otop, t1[:, 0:n], t2[:, 0:n])
                nc.gpsimd.tensor_add(obot, t3[:, 0:n], t4[:, 0:n])

            with ExitStack() as es1:
                vt_ps = es1.enter_context(
                    tc.tile_pool(name="vtps", bufs=1, space="PSUM")
                )
                kv_ps = es1.enter_context(
                    tc.tile_pool(name="kvps", bufs=1, space="PSUM")
                )
                pK = kv_ps.tile([128, T], F32, tag="pk")
                pV = kv_ps.tile([128, T], F32, tag="pv")

                proj(0, pK)
                # K rope: write into KT at each batch's new-token columns
                for (s0, s1, S, kb, L, sp, cs) in B:
                    d = kb + sp
                    rope(pK[0:64, s0:s1], pK[64:128, s0:s1],
                         coskT, sinkT, s0, s1,
                         KT[0:64, d : d + S], KT[64:128, d : d + S],
                         tag=f"k{s0}")
                proj(1, pV)
                nc.scalar.copy(VTsb[:], pV[:])
                vtp = vt_ps.tile([128, NTB, 128], BF, tag="vt")
                for tb in range(NTB):
                    nc.tensor.transpose(
                        vtp[:, tb, :], VTsb[:, tb * 128 : (tb + 1) * 128],
                        ident_bf[:],
                    )
                for (s0, s1, S, kb, L, sp, cs) in B:
                    tb0 = s0 // 128
                    nb = S // 128
                    blk0 = (kb + sp) // 128
                    nc.scalar.copy(
                        Vnat[:, blk0 : blk0 + nb, :],
                        vtp[:, tb0 : tb0 + nb, :],
                    )

            # ---- per-head: Q proj + rope, attention, AllGather ------------
            with ExitStack() as es2:
                ec2 = es2.enter_context
                q_ps = ec2(tc.tile_pool(name="qps", bufs=2, space="PSUM"))
                sc_ps = ec2(tc.tile_pool(name="scps", bufs=3, space="PSUM"))
                at_ps = ec2(tc.tile_pool(name="atps", bufs=2, space="PSUM"))
                sum_ps = ec2(tc.tile_pool(name="sumps", bufs=1, space="PSUM"))

                def emit_scores(h, b):
                    s0, s1, S, kb, L, sp, cs = B[b]
                    tiles = []
                    for j in range(L // 128):
                        dlo = 128 * j - sp
                        c_lo = max(0, dlo)
                        N = S - c_lo
                        sc = sc_ps.tile([128, 512], F32, tag="sc",
                                        name=f"sc_{h}_{b}_{j}")
                        nc.tensor.matmul(
                            sc[:, 0:N],
                            KT[:, kb + 128 * j : kb + 128 * j + 128],
                            QT4[:, h, s0 + c_lo : s0 + S],
                            start=True, stop=True,
                        )
                        if dlo >= 0:
                            m = min(128, S - dlo)
                            nc.vector.tensor_add(
                                sc[:, 0:m], sc[:, 0:m], triT[:, 0:m]
                            )
                        ex = ex_pool.tile([128, 512], BF, tag="ex",
                                          name=f"ex_{h}_{b}_{j}")
                        nc.scalar.activation(
                            ex[:, 0:N], sc[:, 0:N], AFT.Exp,
                            bias=0.0, scale=1.0,
                        )
                        tiles.append((j, c_lo, N, ex))
                    return tiles

                def emit_pv(h, b, tiles, atps, sums):
                    # atps/sums are single-bank [*,512] tiles per token
                    # half; col = token - half_base
                    s0, s1, S, kb, L, sp, cs = B[b]
                    base = 0 if s1 <= 512 else 512
                    nj = len(tiles)
                    for idx, (j, c_lo, N, ex) in enumerate(tiles):
                        st, sp_ = (idx == 0), (idx == nj - 1)
                        nc.tensor.matmul(
                            atps[:, s0 - base + c_lo : s0 - base + S],
                            Vnat[:, kb // 128 + j, :],
                            ex[:, 0:N], start=st, stop=sp_,
                        )
                        nc.tensor.matmul(
                            sums[0:1, s0 - base + c_lo : s0 - base + S],
                            ones_kv[:, 0:1],
                            ex[:, 0:N],
                            start=st, stop=sp_,
                        )

                def emit_attn(h):
                    state = {}

                    def get_half(half):
                        if half not in state:
                            state[half] = (
                                at_ps.tile([128, 512], F32, tag="at",
                                           name=f"at_{h}_{half}"),
                                sum_ps.tile([1, 512], F32, tag="sums",
                                            name=f"sums_{h}_{half}"),
                            )
                        return state[half]

                    def normalize(half):
                        # attnT[:, c] *= 1/sums[c] via PE broadcast
                        atps, sums = state[half]
                        c0, c1 = half * 512, (half + 1) * 512
                        sumsb = st_pool.tile([1, 512], BF, tag="sumsb",
                                             name=f"sumsb_{h}_{half}")
                        nc.scalar.copy(sumsb[:], sums[:])
                        sB = sc_ps.tile([128, 512], F32, tag="sc",
                                        name=f"sB_{h}_{half}")
                        nc.tensor.matmul(
                            sB[:], onesb[:], sumsb[:],
                            start=True, stop=True,
                        )
                        rinv = st_pool.tile([128, 512], F32, tag="rinv",
                                            name=f"rinv_{h}_{half}")
                        nc.vector.reciprocal_approx_fast(rinv[:], sB[:])
                        nc.vector.tensor_mul(
                            attnT_sb[:, h, c0:c1], atps[:], rinv[:]
                        )

                    halfA = [b for b in range(NB) if B[b][1] <= 512]
                    assert halfA and all(
                        B[b][0] >= 512 for b in range(NB) if b not in halfA
                    ), "batches must not straddle the 512-token boundary"

                    def pv_for(b, tiles):
                        half = 0 if B[b][1] <= 512 else 1
                        atps, sums = get_half(half)
                        emit_pv(h, b, tiles, atps, sums)
                        if b == halfA[-1]:
                            normalize(0)

                    prev = None
                    for b in range(NB):
                        tiles = emit_scores(h, b)
                        if prev is not None:
                            pv_for(*prev)
                        prev = (b, tiles)
                    pv_for(*prev)
                    normalize(1)

                    agi = dramb.tile([128, T], BF, name=f"agi{h}")
                    nc.scalar.dma_start(agi[:], attnT_sb[:, h, :])
                    nc.gpsimd.collective_compute(
                        "AllGather",
                        mybir.AluOpType.bypass,
                        replica_groups=[list(range(N_CORES))],
                        ins=[agi.opt()],
                        outs=[ag_out[h][:]],
                    )

                # software-pipelined: Qproj/rope of head h overlap engine
                # work of head h-1's attention/normalize; weight DMAs issued
                # one head ahead
                def load_wq(h):
                    cb = 2 + h
                    wsb[cb] = w_pool.tile([128, KCH, 128], BF, tag="w",
                                          name=f"w_{cb}")
                    nc.sync.dma_start(
                        wsb[cb][:], wqkv_d[cb].rearrange("k p c -> p k c")
                    )

                for h in range(QH_PER):
                    if h + 1 < QH_PER:
                        load_wq(h + 1)
                    for qh in range(2):
                        c0, c1 = qh * 512, (qh + 1) * 512
                        pQ = q_ps.tile([128, 512], F32, tag="pq",
                                       name=f"pq_{h}_{qh}")
                        for k in range(KCH):
                            nc.tensor.matmul(
                                pQ[:],
                                wsb[2 + h][:, k, :],
                                xTsb[:, k, c0:c1],
                                start=(k == 0),
                                stop=(k == KCH - 1),
                            )
                        rope(pQ[0:64, :], pQ[64:128, :],
                             cosqT, sinqT, c0, c1,
                             QT4[0:64, h, c0:c1], QT4[64:128, h, c0:c1],
                             tag=f"q{h}_{qh}")
                    if h > 0:
                        emit_attn(h - 1)
                emit_attn(QH_PER - 1)

            # ---- WO (column shard), consume gathers as they land ----------
            with ExitStack() as es3:
                ec3 = es3.enter_context
                af_pool = ec3(tc.tile_pool(name="af", bufs=3))
                wos_pool = ec3(tc.tile_pool(name="wos", bufs=1))
                osb_pool = ec3(tc.tile_pool(name="osb", bufs=2))
                wo_ps = ec3(tc.tile_pool(name="wops", bufs=1, space="PSUM"))
                wosb = wos_pool.tile([128, H, 512], BF)
                nc.sync.dma_start(wosb[:], wo_d.rearrange("i p c -> p i c"))
                pso = [
                    wo_ps.tile([128, T], F32, tag=f"o{ocb}",
                               name=f"wops_{ocb}")
                    for ocb in range(4)
                ]
                for i in range(H):
                    h, r = i // N_CORES, i % N_CORES
                    af = af_pool.tile([128, T], BF, tag="af",
                                      name=f"af_{i}")
                    nc.sync.dma_start(
                        af[:], ag_out[h][r * 128 : (r + 1) * 128, :]
                    )
                    for ocb in range(4):
                        for tt in range(2):
                            nc.tensor.matmul(
                                pso[ocb][:, tt * 512 : (tt + 1) * 512],
                                wosb[:, i, ocb * 128 : (ocb + 1) * 128],
                                af[:, tt * 512 : (tt + 1) * 512],
                                start=(i == 0),
                                stop=(i == H - 1),
                            )
                for ocb in range(4):
                    ob = osb_pool.tile([128, T], BF, tag="ob",
                                       name=f"ob_{ocb}")
                    nc.vector.tensor_copy(ob[:], pso[ocb][:])
                    nc.sync.dma_start(
                        outT_d[ocb * 128 : (ocb + 1) * 128, :], ob[:]
                    )

    nc.compile()
    return nc


def make_inputs(x, wqkv, wo, kv_cache, seqstarts, kvstarts, cachestarts,
                start_pos):
    """Host-side sharding: per-core input maps (bf16, pre-transposed)."""
    import ml_dtypes

    bf16 = ml_dtypes.bfloat16
    x = np.asarray(x, dtype=np.float32)
    wqkv = np.asarray(wqkv, dtype=np.float32)
    wo = np.asarray(wo, dtype=np.float32)
    kv_cache = np.asarray(kv_cache, dtype=np.float32)
    seqstarts = np.asarray(seqstarts)
    start_pos = np.asarray(start_pos)

    perm = np.concatenate([np.arange(0, HD, 2), np.arange(1, HD, 2)])
    xT = np.ascontiguousarray(x.T).astype(bf16)

    tok = np.arange(T)
    bq = np.clip(
        np.searchsorted(seqstarts, tok, side="right") - 1, 0,
        len(start_pos) - 1,
    )
    pos_q = tok - seqstarts[bq] + start_pos[bq]
    inv_freq = 1.0 / (ROPE_THETA ** (np.arange(D2, dtype=np.float64) / D2))
    ang = inv_freq[:, None] * pos_q[None, :].astype(np.float64)  # [64, T]
    cos = np.cos(ang).astype(np.float32)
    sin = np.sin(ang).astype(np.float32)
    s = np.float32(SCALE)
    top = np.concatenate([cos * s, sin * s, cos, sin], axis=1)  # [64, 4T]
    body = np.concatenate([top, np.zeros_like(top)], axis=0)    # [128, 4T]
    triT = np.where(
        np.arange(128)[:, None] > np.arange(128)[None, :], NEG, 0.0
    ).astype(np.float32)
    consts = np.concatenate([body, triT], axis=1).astype(bf16)

    in_maps = []
    for c in range(N_CORES):
        # packed wqkv col-blocks: [K(perm), V, Q0..Q3(perm)] -> [6,KCH,128,128]
        cols = []
        kc = wqkv[:, HIDDEN + c * HD : HIDDEN + (c + 1) * HD]
        cols.append(kc[:, perm])
        cols.append(wqkv[:, HIDDEN + KVH * HD + c * HD
                         : HIDDEN + KVH * HD + (c + 1) * HD])
        for h in range(QH_PER):
            qh = 4 * c + h
            qc = wqkv[:, qh * HD : (qh + 1) * HD]
            cols.append(qc[:, perm])
        wq = np.stack(cols, 0)                       # [6, HIDDEN, 128]
        wqkv_p = np.ascontiguousarray(
            wq.reshape(6, KCH, 128, 128)
        ).astype(bf16)

        # wo rows in kernel order i=(h,r): global head 4r+h, this core's cols
        wo_c = wo[:, 512 * c : 512 * (c + 1)]
        wo_p = np.empty((H, 128, 512), np.float32)
        for i in range(H):
            h, r = i // N_CORES, i % N_CORES
            g = 4 * r + h
            wo_p[i] = wo_c[g * 128 : (g + 1) * 128, :]
        wo_p = wo_p.astype(bf16)

        ckT = np.ascontiguousarray(
            kv_cache[0, 0][:, c, :].T[perm]
        ).astype(bf16)                               # [128, 8192]
        cv = np.ascontiguousarray(kv_cache[0, 1][:, c, :]).astype(bf16)

        in_maps.append(dict(xT=xT, wqkv_p=wqkv_p, wo_p=wo_p, ckT=ckT,
                            cv=cv, consts=consts))
    return in_maps


_NC_CACHE = {}


def _get_nc(key, seqstarts, kvstarts, cachestarts, start_pos):
    if key not in _NC_CACHE:
        _NC_CACHE[key] = build_nc(seqstarts, kvstarts, cachestarts, start_pos)
    return _NC_CACHE[key]


def run(inputs, trace=False, tmpdir=None):
    """Build (cached), run on 8 cores, return (full_output, results)."""
    seqstarts = np.asarray(inputs["seqstarts"]).tolist()
    kvstarts = np.asarray(inputs["kvstarts"]).tolist()
    cachestarts = np.asarray(inputs["cachestarts"]).tolist()
    start_pos = np.asarray(inputs["start_pos"]).tolist()
    key = tuple(seqstarts) + tuple(kvstarts) + tuple(cachestarts) + tuple(
        start_pos
    )
    nc = _get_nc(key, seqstarts, kvstarts, cachestarts, start_pos)
    in_maps = make_inputs(
        inputs["x"], inputs["wqkv"], inputs["wo"], inputs["kv_cache"],
        seqstarts, kvstarts, cachestarts, start_pos,
    )
    kw = {}
    if trace:
        kw = dict(trace=True, tmpdir=tmpdir)
    res = run_bass_kernel_spmd(nc, in_maps, list(range(N_CORES)), **kw)
    out = np.empty((T, HIDDEN), dtype=np.float32)
    for c in range(N_CORES):
        out[:, 512 * c : 512 * (c + 1)] = (
            res.results[c]["outT"].astype(np.float32).T
        )
    return out, res


def kernel(**inputs) -> np.ndarray:
    out, _ = run(inputs)
    return out
